# revision 1
# baseline (speedup 1.0000x reference)
"""Trainium2 Bass kernel for DenseCRFLoss.

Computes  loss = WEIGHT * (-1/B) * sum_b  sum_{k,i,j} S[b,k,i] K_b[i,j] S[b,k,j]
where K_b[i,j] = exp(-0.5*||f_i - f_j||^2) is the joint bilateral kernel over
downsampled positions+colors (P = 96*96 = 9216 pixels per image).

v3 strategy (cost model: 203.6us (v2 baseline) -> 125.2us; HW rel err 1.3e-3):
  * K is symmetric: only the upper block-triangle is computed. Each (image,
    512-wide block-column J) is cut into pair-range pieces best-fit packed
    into 13 fixed slot capacities SLOTS (sum 173 pairs = 346 [128x512] tiles
    per core, all 8 cores run the same SPMD program; unused tails are
    zero-sT dummies). As[128,512] PSUM accumulates S^T-weighted exp tiles
    over a whole slot, so the sj-epilogue runs once per slot. The triangle
    weights (2 off-diag / 1 diag-band) are folded into the host-side sT.
  * MM1 and MM2 run in fp8e4m3 with MatmulPerfMode.DoubleRow (0.5 PE
    cycles/row vs bf16's 1.0). MM1 contracts 36 rows (hi/mid/lo fp8 splits
    of the 5 features in 6 product groups + 3-term splits of msq/4 against
    const-4 rows, both sides) as 2x18 subtiles: dot = -0.5*d2(i,j) to ~1e-4
    loss error. ISA NOTE: DoubleRow requires col_grp=0xf (stationary M=128;
    MM2's 21-row S^T is zero-padded to 128) and pair-dim step % 16B == 0.
  * exp is computed as C*exp(dot) (C=64 keeps small kernel values above the
    fp8 subnormal floor) on TWO engines concurrently: the scalar engine
    (table exp, bias=ln C, fp8 out) and the vector engine via a Schraudolph
    bit trick -- uint8 = round(dot*8/ln2 + (104-sigma)) saturates at [0,255]
    and IS the fp8e4m3 bit pattern of C*exp(dot). (HW-verified: the DVE
    f32->uint8 convert rounds to nearest and saturates; CoreSim diverges.)
    Pair-groups are split ~89:84 between ACT and DVE by modeled time.
  * Per-slot epilogue: DVE multiply (As[0:21] * sj, 2/C pre-folded into sj)
    + scalar-engine Copy with accum_out for the free-dim reduce (the fused
    tensor_tensor_reduce op crashes real HW). Host sums the partials.
"""

import numpy as np
import ml_dtypes
from contextlib import ExitStack

import concourse.bass as bass
from concourse import bacc
import concourse.tile as tile
from concourse.mybir import (dt, ActivationFunctionType, AluOpType,
                             AxisListType, MatmulPerfMode)
from concourse.bass_utils import run_bass_kernel_spmd

# ---- problem constants (hardcoded; kernel.py must be self-contained) ----
B = 4
KCH = 21
HH = 96                   # downsampled H=W
P = HH * HH               # 9216 pixels
NCORES = 8
SIGMA_RGB = 15.0
SXY_EFF = 100.0 * 0.5     # sigma_xy * scale_factor
WEIGHT = 2e-9

# ---- v3 tuning constants ----
MM1K = 18                  # contraction rows per DoubleRow subtile (2x18 = 36)
C_SCALE = 64.0
LN_C = float(np.log(C_SCALE))
A8 = float(8.0 / np.log(2.0))
SIGMA_SCH = 0.415           # schraudolph bias tuning (see emul2.py scan)
B8 = 56.0 + 8.0 * float(np.log2(C_SCALE)) - SIGMA_SCH
SLOTS = [36, 30, 26, 22, 18, 16, 8, 6, 4, 2, 2, 2, 1]  # pair capacities (sum 173)
NSLOT = len(SLOTS)
NPAIR = sum(SLOTS)         # 180 pairs/core
NTILE3 = 2 * NPAIR         # 360 tiles/core
CACT_NS = 1302.0           # effective (incl. pipeline friction): tuned by sim scan
CDVE_NS = 1282.0           # model: schraudolph instr on DVE per pair group
CEPI_NS = 700.0            # model: per-slot epilogue on DVE
F8NP = ml_dtypes.float8_e4m3

_cache = {}


def _v3_schedule():
    """Deterministic best-fit packing of the 72 (image, column) blocks into
    8 x SLOTS capacities, cutting columns into pair-range pieces when needed
    (partial-column As is fine: the epilogue is linear). Returns per-core
    list over slots of (image, J, start_pair, npairs) or None (all-dummy)."""
    cols = sorted(((2 * (j + 1), b, j) for j in range(18) for b in range(4)),
                  reverse=True)
    free = [(cap, core, idx) for core in range(NCORES)
            for idx, cap in enumerate(SLOTS)]
    free.sort(key=lambda s: (-s[0], s[1], s[2]))
    out = [[None] * NSLOT for _ in range(NCORES)]
    for size, b, j in cols:
        rest = size
        while rest > 0:
            cand = None
            for s in free:          # smallest free slot that fits the rest
                if s[0] >= rest and (cand is None or s[0] < cand[0]):
                    cand = s
            if cand is None:        # split: fill the largest free slot
                assert free, "slot packing failed"
                cand = free[0]
            free.remove(cand)
            take = min(rest, cand[0])
            out[cand[1]][cand[2]] = (b, j, size - rest, take)
            rest -= take
    return out


def _v3_engine_assignment():
    """Greedy ACT/DVE split of the 180 pair-groups by modeled engine time."""
    ends = set(np.cumsum(SLOTS).tolist())
    tA = tD = 0.0
    assign = []
    for g in range(NPAIR):
        if tA <= tD:
            assign.append("A")
            tA += CACT_NS
        else:
            assign.append("D")
            tD += CDVE_NS
        if (g + 1) in ends:
            tD += CEPI_NS
    return assign


def _build_nc_v3(reps=1):
    nc = bacc.Bacc("TRN2", target_bir_lowering=False)
    fD = nc.dram_tensor("fD", [MM1K, 2, NTILE3 * 128], dt.float8e4,
                        kind="ExternalInput")
    gD = nc.dram_tensor("gD", [MM1K, 2, NSLOT * 512], dt.float8e4,
                        kind="ExternalInput")
    sjD = nc.dram_tensor("sjD", [KCH, NSLOT * 512], dt.float32,
                         kind="ExternalInput")
    stD = nc.dram_tensor("stD", [128, 2 * NPAIR, 128], dt.float8e4,
                         kind="ExternalInput")
    out = nc.dram_tensor("out", [KCH, NSLOT], dt.float32, kind="ExternalOutput")
    assign = _v3_engine_assignment()

    with tile.TileContext(nc) as tc, ExitStack() as ctx:
        cpool = ctx.enter_context(tc.tile_pool(name="const", bufs=1))
        accv = cpool.tile([KCH, NSLOT], dt.float32)
        biasc = cpool.tile([128, 1], dt.float32)
        nc.vector.memset(biasc[:], LN_C)

        fpool = ctx.enter_context(tc.tile_pool(name="fstage", bufs=3))
        gpool = ctx.enter_context(tc.tile_pool(name="gstage", bufs=3))
        sjpool = ctx.enter_context(tc.tile_pool(name="sjstage", bufs=3))
        stpool = ctx.enter_context(tc.tile_pool(name="ststage", bufs=3))
        dpool = ctx.enter_context(tc.tile_pool(name="dot", bufs=3, space="PSUM"))
        apool = ctx.enter_context(tc.tile_pool(name="asum", bufs=2, space="PSUM"))
        kpool = ctx.enter_context(tc.tile_pool(name="ktile", bufs=4))
        spool = ctx.enter_context(tc.tile_pool(name="scr", bufs=2))

        for rep in range(reps):
            T = 0
            gpair = 0
            for c, cap in enumerate(SLOTS):
                f_sl = fpool.tile([MM1K, 2, 72 * 128], dt.float8e4, tag="fs")
                nc.sync.dma_start(f_sl[:, :, :cap * 2 * 128],
                                  fD[:, :, T * 128:(T + 2 * cap) * 128])
                g_sl = gpool.tile([MM1K, 2, 512], dt.float8e4, tag="gs")
                nc.sync.dma_start(g_sl[:], gD[:, :, c * 512:(c + 1) * 512])
                st_sl = stpool.tile([128, 72, 128], dt.float8e4, tag="sts")
                nc.sync.dma_start(st_sl[:, :2 * cap, :], stD[:, T:T + 2 * cap, :])
                sj_sl = sjpool.tile([KCH, 512], dt.float32, tag="sjs")
                nc.sync.dma_start(sj_sl[:], sjD[:, c * 512:(c + 1) * 512])
                As = apool.tile([128, 512], dt.float32, tag="As", name="As")
                for p in range(cap):
                    dot = dpool.tile([128, 2, 512], dt.float32, tag="dot")
                    for h in range(2):
                        t_loc = 2 * p + h
                        nc.tensor.matmul(
                            dot[:, h, :],
                            f_sl[:, :, t_loc * 128:(t_loc + 1) * 128],
                            g_sl[:],
                            start=True, stop=True,
                            perf_mode=MatmulPerfMode.DoubleRow)
                    kt = kpool.tile([128, 2, 512], dt.float8e4, tag="kt",
                                    name="kt")
                    if assign[gpair] == "A":
                        nc.scalar.activation(kt[:], dot[:],
                                             ActivationFunctionType.Exp,
                                             bias=biasc[:])
                    else:
                        nc.vector.tensor_scalar(
                            kt[:].bitcast(dt.uint8), dot[:], A8, B8,
                            AluOpType.mult, AluOpType.add)
                    nc.tensor.matmul(
                        As[:], st_sl[:, 2 * p:2 * p + 2, :], kt[:],
                        start=(p == 0), stop=(p == cap - 1),
                        perf_mode=MatmulPerfMode.DoubleRow)
                    T += 2
                    gpair += 1
                scr = spool.tile([KCH, 512], dt.float32, tag="scr")
                nc.vector.tensor_mul(scr[:], As[0:KCH, :], sj_sl[:])
                scr2 = spool.tile([KCH, 512], dt.float32, tag="scr2")
                nc.scalar.activation(scr2[:], scr[:],
                                     ActivationFunctionType.Copy,
                                     accum_out=accv[:, c:c + 1])
        nc.sync.dma_start(out[:], accv[:])
    nc.finalize()
    return nc


def _split3_f8(x):
    h = x.astype(F8NP)
    r = x - h.astype(np.float32)
    m = r.astype(F8NP)
    l = (r - m.astype(np.float32)).astype(F8NP)
    return (h.astype(np.float32), m.astype(np.float32), l.astype(np.float32))


def _prep_inputs_v3(segmentations, images):
    seg = np.asarray(segmentations, dtype=np.float32)
    img = np.asarray(images, dtype=np.float32)
    S = seg.reshape(B, KCH, HH, 2, HH, 2).mean(axis=(3, 5)).reshape(B, KCH, P)
    rgb = img[:, :, ::2, ::2].reshape(B, 3, P)
    yy, xx = np.meshgrid(np.arange(HH, dtype=np.float32),
                         np.arange(HH, dtype=np.float32), indexing="ij")
    pos = np.stack([xx.ravel(), yy.ravel()], axis=0) / SXY_EFF

    fS, gS = [], []
    for b in range(B):
        feat = np.concatenate([pos, rgb[b] / SIGMA_RGB], axis=0).astype(np.float32)
        msq = (-0.5 * (feat.astype(np.float64) ** 2).sum(axis=0)).astype(np.float32)
        Fh, Fm, Fl = _split3_f8(feat)
        qh, qm, ql = _split3_f8(msq / 4.0)
        c4 = np.full((3, P), 4.0, np.float32)
        q3 = np.concatenate([qh[None], qm[None], ql[None]], axis=0)
        fA = np.concatenate([Fh, Fh, Fm, c4], axis=0)          # [18, P]
        fB = np.concatenate([Fh, Fl, Fm, q3], axis=0)
        gA = np.concatenate([Fh, Fm, Fh, q3], axis=0)
        gB_ = np.concatenate([Fl, Fh, Fm, c4], axis=0)
        fS.append(np.stack([fA, fB], axis=1).astype(F8NP))     # [18, 2, P]
        gS.append(np.stack([gA, gB_], axis=1).astype(F8NP))

    sched = _v3_schedule()
    in_maps = []
    for core in range(NCORES):
        fDa = np.zeros((MM1K, 2, NTILE3 * 128), F8NP)
        gDa = np.zeros((MM1K, 2, NSLOT * 512), F8NP)
        sjDa = np.zeros((KCH, NSLOT * 512), np.float32)
        stDa = np.zeros((128, 2 * NPAIR, 128), F8NP)
        T = 0
        for c, ent in enumerate(sched[core]):
            cap = SLOTS[c]
            if ent is None:
                T += 2 * cap
                continue
            b, J, start, npairs = ent
            gDa[:, :, c * 512:(c + 1) * 512] = gS[b][:, :, J * 512:(J + 1) * 512]
            sjDa[:, c * 512:(c + 1) * 512] = (
                S[b][:, J * 512:(J + 1) * 512] * np.float32(2.0 / C_SCALE))
            for p in range(cap):
                for h in range(2):
                    q = 2 * (start + p) + h
                    if p < npairs:
                        fDa[:, :, T * 128:(T + 1) * 128] = (
                            fS[b][:, :, q * 128:(q + 1) * 128])
                        w = np.float32(0.5 if q >= 4 * J else 1.0)
                        stDa[:, T, :KCH] = (
                            w * S[b][:, q * 128:(q + 1) * 128].T).astype(F8NP)
                    T += 1
        in_maps.append({"fD": fDa, "gD": gDa, "sjD": sjDa, "stD": stDa})
    return in_maps


def kernel(segmentations, images, _trace=False):
    key = "v3"
    if key not in _cache:
        _cache[key] = _build_nc_v3()
    nc = _cache[key]
    in_maps = _prep_inputs_v3(segmentations, images)
    res = run_bass_kernel_spmd(nc, in_maps, core_ids=list(range(NCORES)),
                               trace=_trace)
    kernel._last_results = res
    total = sum(float(np.asarray(r["out"], dtype=np.float64).sum())
                for r in res.results)
    return np.asarray(np.float32(-WEIGHT * total / B))


def _make_timer(nc, in_maps, timing_reps):
    """Build the jitted SPMD executor for `nc` (mirrors
    bass2jax.run_bass_via_pjrt multi-core path) with device-resident inputs;
    return min wall-clock ns over `timing_reps` calls."""
    import time
    import jax
    from jax.sharding import Mesh, PartitionSpec, NamedSharding
    from jax.experimental.shard_map import shard_map
    import concourse.mybir as mybir
    from concourse import bass2jax

    bass2jax.install_neuronx_cc_hook()
    partition_name = nc.partition_id_tensor.name if nc.partition_id_tensor else None
    in_names, out_names, out_avals, zero_outs = [], [], [], []
    for alloc in nc.m.functions[0].allocations:
        if not isinstance(alloc, mybir.MemoryLocationSet):
            continue
        name = alloc.memorylocations[0].name
        if alloc.kind == "ExternalInput":
            if name != partition_name:
                in_names.append(name)
        elif alloc.kind == "ExternalOutput":
            out_names.append(name)
            shape = tuple(alloc.tensor_shape)
            dtype = mybir.dt.np(alloc.dtype)
            out_avals.append(jax.core.ShapedArray(shape, dtype))
            zero_outs.append(np.zeros(shape, dtype))
    n_params = len(in_names)

    def _body(*args):
        operands = list(args)
        if partition_name is not None:
            operands.append(bass2jax.partition_id_tensor())
        outs = bass2jax._bass_exec_p.bind(
            *operands,
            out_avals=tuple(out_avals),
            in_names=tuple(in_names + out_names
                           + ([partition_name] if partition_name else [])),
            out_names=tuple(out_names),
            lowering_input_output_aliases=(),
            sim_require_finite=True,
            sim_require_nnan=True,
            nc=nc,
        )
        return tuple(outs)

    devices = jax.devices()[:NCORES]
    mesh = Mesh(np.asarray(devices), ("core",))
    in_specs = (PartitionSpec("core"),) * (n_params + len(out_names))
    out_specs = (PartitionSpec("core"),) * len(out_names)
    sharded = jax.jit(
        shard_map(_body, mesh=mesh, in_specs=in_specs, out_specs=out_specs,
                  check_rep=False),
        keep_unused=True,
    )
    per_core = [[np.asarray(m[name]) for name in in_names] for m in in_maps]
    concat_in = [
        jax.device_put(
            np.concatenate([per_core[c][i] for c in range(NCORES)], axis=0),
            NamedSharding(mesh, PartitionSpec("core")))
        for i in range(n_params)
    ]
    concat_zeros = [
        jax.device_put(np.zeros((NCORES * z.shape[0], *z.shape[1:]), z.dtype),
                       NamedSharding(mesh, PartitionSpec("core")))
        for z in zero_outs
    ]
    out = sharded(*concat_in, *concat_zeros)  # compile + warm
    jax.block_until_ready(out)

    def sample(n):
        best = float("inf")
        for _ in range(n):
            t0 = time.perf_counter_ns()
            jax.block_until_ready(sharded(*concat_in, *concat_zeros))
            best = min(best, time.perf_counter_ns() - t0)
        return best

    sample(2)  # extra warm
    return sample


def benchmark(segmentations, images, reps=60, r_hi=21):
    """Estimate on-device kernel time via the replication slope: build the
    kernel with the main loop repeated 1x and r_hi times, take
    (t(r_hi) - t(1)) / (r_hi - 1). The ~100 ms axon tunnel round-trip
    cancels in the difference."""
    in_maps = _prep_inputs_v3(segmentations, images)
    timers = {r: _make_timer(_build_nc_v3(reps=r), in_maps, reps)
              for r in (1, r_hi)}
    # Interleave the two configurations in time: the axon tunnel RTT drifts
    # by ~2x over minutes, which swamps a single t(r_hi)-t(1) difference.
    # Take 3 paired samples; the caller can demand they agree before
    # trusting wall clock over the cost model.
    slopes = []
    times = []
    for _ in range(3):
        t1 = timers[1](reps // 3 + 1)
        th = timers[r_hi](reps // 3 + 1)
        slopes.append((th - t1) / (r_hi - 1))
        times.append((t1, th))
    benchmark._last = times
    benchmark._slopes = slopes
    return float(np.median(slopes))

